# revision 2
# baseline (speedup 1.0000x reference)
"""AUGRU cell kernel for Trainium2 (Bass/Tile), data-parallel over 8 NeuronCores.

Computes, for full inputs [B=32768, 512]:
    u = sigmoid(x @ Wu_x + bu + h @ Wu_h)
    r = sigmoid(x @ Wr_x + br + h @ Wr_h)
    c = tanh(x @ Wc_x + bc + r * (h @ Wc_h))
    u_ = att * u
    out = (1 - u_) * h + u_ * c

Sharding: batch dim split 8 ways (4096 rows/core); the six 512x512 weight
matrices are replicated to every core.

Per-core kernel structure (32 tiles of 128 batch rows):
  - weights preloaded to SBUF as [128, 4, 512] (K-chunked), dtype float32r
  - per tile: PE-transpose x/h tiles ([128,512] -> 4x 128x128 transposes each)
    into PSUM, copy to SBUF, then 24 float32r matmuls (K=1024 fused u|r into
    one 2-bank PSUM tile, plus c_x and c_h groups), ACT sigmoid/tanh + DVE
    elementwise epilogue, DMA out.
  - float32r (FP22-truncated fp32 multiply, fp32 accumulate) runs the PE at
    1 col/cycle like bf16 but with ~2^-14 relative precision. walrus requires
    the whole producer chain of matmul operands to be float32r, so x/h/weights
    are declared float32r end to end; epilogue reads bitcast back to f32.
"""

import sys

import numpy as np

if "/opt/trn_rl_repo" not in sys.path:
    sys.path.insert(0, "/opt/trn_rl_repo")

B = 32768
D = 512
U = 512
NCORES = 8
BLOC = B // NCORES  # 4096
P = 128
NT = BLOC // P  # 32
KX = D // P  # 4
KH = U // P  # 4

_cache = {}


def _build(with_bias: bool):
    import concourse.bacc as bacc
    import concourse.mybir as mybir
    from concourse.tile import TileContext

    f32 = mybir.dt.float32
    f32r = mybir.dt.float32r
    Alu = mybir.AluOpType
    Act = mybir.ActivationFunctionType

    nc = bacc.Bacc(None, target_bir_lowering=False)

    x_d = nc.dram_tensor("x", [BLOC, D], f32r, kind="ExternalInput")
    h_d = nc.dram_tensor("h", [BLOC, U], f32r, kind="ExternalInput")
    a_d = nc.dram_tensor("att", [BLOC, 1], f32, kind="ExternalInput")
    i_d = nc.dram_tensor("ident", [P, P], f32r, kind="ExternalInput")
    w_names = ["wux", "wuh", "wrx", "wrh", "wcx", "wch"]
    w_d = {n: nc.dram_tensor(n, [D, U], f32r, kind="ExternalInput") for n in w_names}
    b_d = {}
    if with_bias:
        # bias broadcast is done with a K=1 matmul: ones[1,128].T @ bias[1,512]
        b_d["ones"] = nc.dram_tensor("ones", [1, P], f32r, kind="ExternalInput")
        for n in ["bu", "br", "bc"]:
            b_d[n] = nc.dram_tensor(n, [1, U], f32r, kind="ExternalInput")
    o_d = nc.dram_tensor("out", [BLOC, U], f32, kind="ExternalOutput")

    with TileContext(nc) as tc:
        with (
            tc.tile_pool(name="wpool", bufs=1) as wpool,
            tc.tile_pool(name="xin", bufs=4) as xin_pool,
            tc.tile_pool(name="hin", bufs=4) as hin_pool,
            tc.tile_pool(name="xht", bufs=2) as xht_pool,
            tc.tile_pool(name="ep", bufs=3) as ep_pool,
            tc.tile_pool(name="ptr", bufs=4, space="PSUM") as ptr_pool,
            tc.tile_pool(name="pur", bufs=1, space="PSUM") as pur_pool,
            tc.tile_pool(name="pc", bufs=1, space="PSUM") as pc_pool,
        ):
            # ---- one-time preloads ----
            w_sb = {}
            for n in w_names:
                t = wpool.tile([P, 4, U], f32r, tag=n)
                nc.sync.dma_start(t[:], w_d[n].rearrange("(ko p) n -> p ko n", p=P))
                w_sb[n] = t
            ident = wpool.tile([P, P], f32r, tag="ident")
            nc.sync.dma_start(ident[:], i_d[:, :])
            att_all = wpool.tile([P, NT], f32, tag="attall")
            nc.sync.dma_start(att_all[:], a_d.rearrange("(t p) o -> p (t o)", p=P))

            ones_sb = None
            bias_sb = {}
            if with_bias:
                ones_sb = wpool.tile([1, P], f32r, tag="ones")
                nc.sync.dma_start(ones_sb[:], b_d["ones"][:, :])
                for n in ["bu", "br", "bc"]:
                    t = wpool.tile([1, U], f32r, tag=n)
                    nc.sync.dma_start(t[:], b_d[n][:, :])
                    bias_sb[n] = t

            xcols = [slice(j * P, (j + 1) * P) for j in range(KX)]
            hcols = [slice(D + j * P, D + (j + 1) * P) for j in range(KH)]

            def acc_group(psum_slice, xhT, terms, bias_tile):
                """Accumulate sum of lhsT.T @ rhs terms (+ bias broadcast) into
                one PSUM bank via float32r matmuls."""
                n_mm = len(terms) + (1 if bias_tile is not None else 0)
                idx = 0
                if bias_tile is not None:
                    nc.tensor.matmul(
                        psum_slice,
                        ones_sb[:, :],
                        bias_tile[:, :],
                        start=True,
                        stop=(n_mm == 1),
                    )
                    idx = 1
                for cols, rhs_ap in terms:
                    nc.tensor.matmul(
                        psum_slice,
                        xhT[:, cols],
                        rhs_ap,
                        start=(idx == 0),
                        stop=(idx == n_mm - 1),
                    )
                    idx += 1

            stage = [None] * NT
            for i in range(NT + 1):
                if i < NT:
                    # ---- stage A: load + transpose tile i (runs on PE ahead
                    # of tile i-1's matmuls to hide the PSUM->SBUF copies) ----
                    rows = slice(i * P, (i + 1) * P)
                    xt = xin_pool.tile([P, D], f32r, tag="x")
                    nc.sync.dma_start(xt[:], x_d[rows, :])
                    ht = hin_pool.tile([P, U], f32r, tag="h")
                    nc.sync.dma_start(ht[:], h_d[rows, :])
                    xT_ps = ptr_pool.tile([P, D], f32r, tag="tr")
                    hT_ps = ptr_pool.tile([P, U], f32r, tag="tr")
                    for j in range(KX):
                        nc.tensor.transpose(
                            xT_ps[:, xcols[j]], xt[:, xcols[j]], ident[:]
                        )
                    for j in range(KH):
                        nc.tensor.transpose(
                            hT_ps[:, xcols[j]], ht[:, xcols[j]], ident[:]
                        )
                    xhT = xht_pool.tile([P, D + U], f32r, tag="xhT")
                    nc.vector.tensor_copy(xhT[:, 0:D], xT_ps[:])
                    nc.vector.tensor_copy(xhT[:, D : D + U], hT_ps[:])
                    stage[i] = (xt, ht, xhT)
                if i >= 1:
                    # ---- stage B: matmuls + epilogue for tile i-1 ----
                    ii = i - 1
                    xt, ht, xhT = stage[ii]
                    stage[ii] = None
                    ht_f32 = ht[:].bitcast(f32)
                    p_ur = pur_pool.tile([P, 2 * U], f32, tag="ur")
                    p_c = pc_pool.tile([P, 2 * U], f32, tag="c")

                    # u gate: x@Wu_x + h@Wu_h (+bu)
                    acc_group(
                        p_ur[:, 0:U],
                        xhT,
                        [(xcols[j], w_sb["wux"][:, j, :]) for j in range(KX)]
                        + [(hcols[j], w_sb["wuh"][:, j, :]) for j in range(KH)],
                        bias_sb.get("bu"),
                    )
                    # r gate
                    acc_group(
                        p_ur[:, U : 2 * U],
                        xhT,
                        [(xcols[j], w_sb["wrx"][:, j, :]) for j in range(KX)]
                        + [(hcols[j], w_sb["wrh"][:, j, :]) for j in range(KH)],
                        bias_sb.get("br"),
                    )
                    # c_h = h @ Wc_h  (first, so r*c_h can start early)
                    acc_group(
                        p_c[:, U : 2 * U],
                        xhT,
                        [(hcols[j], w_sb["wch"][:, j, :]) for j in range(KH)],
                        None,
                    )
                    # c_x = x @ Wc_x (+bc)
                    acc_group(
                        p_c[:, 0:U],
                        xhT,
                        [(xcols[j], w_sb["wcx"][:, j, :]) for j in range(KX)],
                        bias_sb.get("bc"),
                    )

                    u_sb = ep_pool.tile([P, U], f32, tag="u")
                    r_sb = ep_pool.tile([P, U], f32, tag="r")
                    nc.scalar.activation(u_sb[:], p_ur[:, 0:U], Act.Sigmoid)
                    nc.scalar.activation(r_sb[:], p_ur[:, U : 2 * U], Act.Sigmoid)
                    m_sb = ep_pool.tile([P, U], f32, tag="m")
                    # m = c_x + r * c_h
                    nc.vector.tensor_tensor(
                        m_sb[:], r_sb[:], p_c[:, U : 2 * U], Alu.mult
                    )
                    nc.vector.tensor_tensor(m_sb[:], m_sb[:], p_c[:, 0:U], Alu.add)
                    c_sb = ep_pool.tile([P, U], f32, tag="c")
                    nc.scalar.activation(c_sb[:], m_sb[:], Act.Tanh)
                    # out = h + (att*u) * (c - h)
                    nc.vector.tensor_tensor(c_sb[:], c_sb[:], ht_f32, Alu.subtract)
                    nc.vector.tensor_tensor(c_sb[:], u_sb[:], c_sb[:], Alu.mult)
                    o_sb = ep_pool.tile([P, U], f32, tag="o")
                    nc.vector.scalar_tensor_tensor(
                        o_sb[:],
                        c_sb[:],
                        att_all[:, ii : ii + 1],
                        ht_f32,
                        Alu.mult,
                        Alu.add,
                    )
                    nc.sync.dma_start(o_d[ii * P : (ii + 1) * P, :], o_sb[:])

    nc.compile()
    return nc


def _get_nc(with_bias: bool):
    key = bool(with_bias)
    if key not in _cache:
        _cache[key] = _build(key)
    return _cache[key]


def _run(inputs, state, att_score, Wu_x, bu, Wu_h, Wr_x, br, Wr_h, Wc_x, bc, Wc_h,
         trace=False):
    from concourse.bass_utils import run_bass_kernel_spmd

    with_bias = bool(np.any(bu) or np.any(br) or np.any(bc))
    nc = _get_nc(with_bias)

    def f32c(a):
        return np.ascontiguousarray(np.asarray(a, dtype=np.float32))

    inputs = f32c(inputs)
    state = f32c(state)
    att_score = f32c(att_score)
    shared = {
        "wux": f32c(Wu_x),
        "wuh": f32c(Wu_h),
        "wrx": f32c(Wr_x),
        "wrh": f32c(Wr_h),
        "wcx": f32c(Wc_x),
        "wch": f32c(Wc_h),
        "ident": np.eye(P, dtype=np.float32),
    }
    if with_bias:
        shared["ones"] = np.ones((1, P), dtype=np.float32)
        shared["bu"] = f32c(bu).reshape(1, U)
        shared["br"] = f32c(br).reshape(1, U)
        shared["bc"] = f32c(bc).reshape(1, U)

    in_maps = []
    for c in range(NCORES):
        sl = slice(c * BLOC, (c + 1) * BLOC)
        m = {
            "x": inputs[sl],
            "h": state[sl],
            "att": att_score[sl],
        }
        m.update(shared)
        in_maps.append(m)

    res = run_bass_kernel_spmd(nc, in_maps, core_ids=list(range(NCORES)), trace=trace)
    out = np.concatenate([r["out"] for r in res.results], axis=0)
    return out, res


def kernel(inputs, state, att_score, Wu_x, bu, Wu_h, Wr_x, br, Wr_h, Wc_x, bc, Wc_h):
    out, _ = _run(
        inputs, state, att_score, Wu_x, bu, Wu_h, Wr_x, br, Wr_h, Wc_x, bc, Wc_h
    )
    return out


# revision 8
# speedup vs baseline: 1.0283x; 1.0283x over previous
"""AUGRU cell kernel for Trainium2 (Bass/Tile), data-parallel over 8 NeuronCores.

Computes, for full inputs [B=32768, 512]:
    u = sigmoid(x @ Wu_x + bu + h @ Wu_h)
    r = sigmoid(x @ Wr_x + br + h @ Wr_h)
    c = tanh(x @ Wc_x + bc + r * (h @ Wc_h))
    u_ = att * u
    out = (1 - u_) * h + u_ * c

Sharding: batch dim split 8 ways (4096 rows/core); the six 512x512 weight
matrices are replicated to every core.

Per-core kernel structure (32 tiles of 128 batch rows):
  - weights preloaded to SBUF as [128, 4, 512] (K-chunked), dtype float32r
  - per tile: PE-transpose x/h tiles ([128,512] -> 4x 128x128 transposes each)
    into PSUM, copy to SBUF, then 24 float32r matmuls (K=1024 fused u|r into one 2-bank PSUM
    tile, plus c_x and c_h groups), ACT sigmoid/tanh + DVE elementwise
    epilogue, DMA out.
  - float32r (FP22-truncated fp32 multiply, fp32 accumulate) runs the PE at
    1 col/cycle like bf16 but with ~2^-14 relative precision. walrus requires
    the whole producer chain of matmul operands to be float32r, so x/h/weights
    are declared float32r end to end; epilogue reads bitcast back to f32.
  - startup: weight DMAs are interleaved with the first four tiles' input
    DMAs in consumption order, so the PE starts transposing ~2us in instead
    of waiting ~30us for all weights.
"""

import sys

import numpy as np

if "/opt/trn_rl_repo" not in sys.path:
    sys.path.insert(0, "/opt/trn_rl_repo")

B = 32768
D = 512
U = 512
NCORES = 8
BLOC = B // NCORES  # 4096
P = 128
NT = BLOC // P  # 32
KX = D // P  # 4
KH = U // P  # 4

_cache = {}


def _build(with_bias: bool):
    import concourse.bacc as bacc
    import concourse.mybir as mybir
    from concourse.tile import TileContext

    f32 = mybir.dt.float32
    f32r = mybir.dt.float32r
    bf16 = mybir.dt.bfloat16
    Alu = mybir.AluOpType
    Act = mybir.ActivationFunctionType

    nc = bacc.Bacc(None, target_bir_lowering=False)

    x_d = nc.dram_tensor("x", [BLOC, D], f32r, kind="ExternalInput")
    h_d = nc.dram_tensor("h", [BLOC, U], f32r, kind="ExternalInput")
    a_d = nc.dram_tensor("att", [BLOC, 1], f32, kind="ExternalInput")
    i_d = nc.dram_tensor("ident", [P, P], f32r, kind="ExternalInput")
    w_names = ["wux", "wuh", "wrx", "wrh", "wcx", "wch"]
    w_d = {n: nc.dram_tensor(n, [D, U], f32r, kind="ExternalInput") for n in w_names}
    b_d = {}
    if with_bias:
        # bias broadcast is done with a K=1 matmul: ones[1,128].T @ bias[1,512]
        b_d["ones"] = nc.dram_tensor("ones", [1, P], f32r, kind="ExternalInput")
        for n in ["bu", "br", "bc"]:
            b_d[n] = nc.dram_tensor(n, [1, U], f32r, kind="ExternalInput")
    o_d = nc.dram_tensor("out", [BLOC, U], f32, kind="ExternalOutput")

    with TileContext(nc) as tc:
        with (
            tc.tile_pool(name="wpool", bufs=1) as wpool,
            tc.tile_pool(name="xin", bufs=6) as xin_pool,
            tc.tile_pool(name="hin", bufs=6) as hin_pool,
            tc.tile_pool(name="xht", bufs=4) as xht_pool,
            tc.tile_pool(name="ep", bufs=3) as ep_pool,
            tc.tile_pool(name="ptr", bufs=4, space="PSUM") as ptr_pool,
            tc.tile_pool(name="pur", bufs=1, space="PSUM") as pur_pool,
            tc.tile_pool(name="pc", bufs=1, space="PSUM") as pc_pool,
        ):
            ident = wpool.tile([P, P], f32r, tag="ident")
            nc.sync.dma_start(ident[:], i_d[:, :])

            w_sb = {
                n: wpool.tile([P, 4, U], f32r, tag=n, name=f"w_{n}") for n in w_names
            }

            def load_w(n):
                nc.sync.dma_start(
                    w_sb[n][:], w_d[n].rearrange("(ko p) n -> p ko n", p=P)
                )

            att_all = wpool.tile([P, NT], f32, tag="attall")

            ones_sb = None
            bias_sb = {}

            xcols = [slice(j * P, (j + 1) * P) for j in range(KX)]
            hcols = [slice(D + j * P, D + (j + 1) * P) for j in range(KH)]

            def acc_group(psum_slice, xhT, terms, bias_tile):
                """Accumulate sum of lhsT.T @ rhs terms (+ bias broadcast) into
                one PSUM bank via float32r matmuls."""
                n_mm = len(terms) + (1 if bias_tile is not None else 0)
                idx = 0
                if bias_tile is not None:
                    nc.tensor.matmul(
                        psum_slice,
                        ones_sb[:, :],
                        bias_tile[:, :],
                        start=True,
                        stop=(n_mm == 1),
                    )
                    idx = 1
                for cols, rhs_ap in terms:
                    nc.tensor.matmul(
                        psum_slice,
                        xhT[:, cols],
                        rhs_ap,
                        start=(idx == 0),
                        stop=(idx == n_mm - 1),
                    )
                    idx += 1

            stage = [None] * NT

            def stage_a(i):
                # load + transpose tile i (emitted ahead of tile i-1's matmuls
                # on the PE so the PSUM->SBUF copies are off the critical path)
                rows = slice(i * P, (i + 1) * P)
                xt = xin_pool.tile([P, D], f32r, tag="x")
                nc.sync.dma_start(xt[:], x_d[rows, :])
                ht = hin_pool.tile([P, U], f32r, tag="h")
                nc.sync.dma_start(ht[:], h_d[rows, :])
                xT_ps = ptr_pool.tile([P, D], f32r, tag="tr")
                hT_ps = ptr_pool.tile([P, U], f32r, tag="tr")
                for j in range(KX):
                    nc.tensor.transpose(xT_ps[:, xcols[j]], xt[:, xcols[j]], ident[:])
                for j in range(KH):
                    nc.tensor.transpose(hT_ps[:, xcols[j]], ht[:, xcols[j]], ident[:])
                xhT = xht_pool.tile([P, D + U], f32r, tag="xhT")
                nc.vector.tensor_copy(xhT[:, 0:D], xT_ps[:])
                nc.vector.tensor_copy(xhT[:, D : D + U], hT_ps[:])
                stage[i] = (xt, ht, xhT)

            def stage_b(ii):
                # matmuls + epilogue for tile ii
                xt, ht, xhT = stage[ii]
                stage[ii] = None
                ht_f32 = ht[:].bitcast(f32)
                p_ur = pur_pool.tile([P, 2 * U], f32, tag="ur")
                p_c = pc_pool.tile([P, 2 * U], f32, tag="c")

                # u gate: x@Wu_x + h@Wu_h (+bu)
                acc_group(
                    p_ur[:, 0:U],
                    xhT,
                    [(xcols[j], w_sb["wux"][:, j, :]) for j in range(KX)]
                    + [(hcols[j], w_sb["wuh"][:, j, :]) for j in range(KH)],
                    bias_sb.get("bu"),
                )
                # r gate
                acc_group(
                    p_ur[:, U : 2 * U],
                    xhT,
                    [(xcols[j], w_sb["wrx"][:, j, :]) for j in range(KX)]
                    + [(hcols[j], w_sb["wrh"][:, j, :]) for j in range(KH)],
                    bias_sb.get("br"),
                )
                # c_h = h @ Wc_h  (before c_x, so r*c_h can start early)
                acc_group(
                    p_c[:, U : 2 * U],
                    xhT,
                    [(hcols[j], w_sb["wch"][:, j, :]) for j in range(KH)],
                    None,
                )
                # c_x = x @ Wc_x (+bc)
                acc_group(
                    p_c[:, 0:U],
                    xhT,
                    [(xcols[j], w_sb["wcx"][:, j, :]) for j in range(KX)],
                    bias_sb.get("bc"),
                )

                u_sb = ep_pool.tile([P, U], f32, tag="u")
                r_sb = ep_pool.tile([P, U], f32, tag="r")
                nc.scalar.activation(u_sb[:], p_ur[:, 0:U], Act.Sigmoid)
                nc.scalar.activation(r_sb[:], p_ur[:, U : 2 * U], Act.Sigmoid)
                m_sb = ep_pool.tile([P, U], f32, tag="m")
                # m = c_x + r * c_h
                nc.vector.tensor_tensor(m_sb[:], r_sb[:], p_c[:, U : 2 * U], Alu.mult)
                nc.vector.tensor_tensor(m_sb[:], m_sb[:], p_c[:, 0:U], Alu.add)
                c_sb = ep_pool.tile([P, U], f32, tag="c")
                nc.scalar.activation(c_sb[:], m_sb[:], Act.Tanh)
                # out = h + (att*u) * (c - h)
                nc.vector.tensor_tensor(c_sb[:], c_sb[:], ht_f32, Alu.subtract)
                nc.vector.tensor_tensor(c_sb[:], u_sb[:], c_sb[:], Alu.mult)
                o_sb = ep_pool.tile([P, U], f32, tag="o")
                nc.vector.scalar_tensor_tensor(
                    o_sb[:],
                    c_sb[:],
                    att_all[:, ii : ii + 1],
                    ht_f32,
                    Alu.mult,
                    Alu.add,
                )
                nc.sync.dma_start(o_d[ii * P : (ii + 1) * P, :], o_sb[:])

            # ---- startup: interleave weight DMAs with the first tiles in
            # consumption order (u needs wux+wuh, r needs wrx+wrh, then wch,
            # wcx), so the PE gets work ~2us in instead of ~30us ----
            stage_a(0)
            load_w("wux")
            load_w("wuh")
            stage_a(1)
            load_w("wrx")
            load_w("wrh")
            stage_a(2)
            load_w("wch")
            load_w("wcx")
            if with_bias:
                ones_sb = wpool.tile([1, P], f32r, tag="ones")
                nc.sync.dma_start(ones_sb[:], b_d["ones"][:, :])
                for n in ["bu", "br", "bc"]:
                    t = wpool.tile([1, U], f32r, tag=n)
                    nc.sync.dma_start(t[:], b_d[n][:, :])
                    bias_sb[n] = t
            nc.sync.dma_start(att_all[:], a_d.rearrange("(t p) o -> p (t o)", p=P))
            stage_b(0)
            stage_a(3)
            stage_b(1)
            for i in range(4, NT):
                stage_a(i)
                stage_b(i - 2)
            stage_b(NT - 2)
            stage_b(NT - 1)

    nc.compile()
    return nc


def _get_nc(with_bias: bool):
    key = bool(with_bias)
    if key not in _cache:
        _cache[key] = _build(key)
    return _cache[key]


def _run(inputs, state, att_score, Wu_x, bu, Wu_h, Wr_x, br, Wr_h, Wc_x, bc, Wc_h,
         trace=False):
    import ml_dtypes
    from concourse.bass_utils import run_bass_kernel_spmd

    with_bias = bool(np.any(bu) or np.any(br) or np.any(bc))
    nc = _get_nc(with_bias)

    def f32c(a):
        return np.ascontiguousarray(np.asarray(a, dtype=np.float32))

    inputs = f32c(inputs)
    state = f32c(state)
    att_score = f32c(att_score)
    shared = {
        "wux": f32c(Wu_x),
        "wuh": f32c(Wu_h),
        "wrx": f32c(Wr_x),
        "wrh": f32c(Wr_h),
        "wcx": f32c(Wc_x),
        "wch": f32c(Wc_h),
        "ident": np.eye(P, dtype=np.float32),
    }
    if with_bias:
        shared["ones"] = np.ones((1, P), dtype=np.float32)
        shared["bu"] = f32c(bu).reshape(1, U)
        shared["br"] = f32c(br).reshape(1, U)
        shared["bc"] = f32c(bc).reshape(1, U)

    in_maps = []
    for c in range(NCORES):
        sl = slice(c * BLOC, (c + 1) * BLOC)
        m = {
            "x": inputs[sl],
            "h": state[sl],
            "att": att_score[sl],
        }
        m.update(shared)
        in_maps.append(m)

    res = run_bass_kernel_spmd(nc, in_maps, core_ids=list(range(NCORES)), trace=trace)
    out = np.concatenate([r["out"] for r in res.results], axis=0)
    return out, res


def kernel(inputs, state, att_score, Wu_x, bu, Wu_h, Wr_x, br, Wr_h, Wc_x, bc, Wc_h):
    out, _ = _run(
        inputs, state, att_score, Wu_x, bu, Wu_h, Wr_x, br, Wr_h, Wc_x, bc, Wc_h
    )
    return out
